# revision 21
# baseline (speedup 1.0000x reference)
"""DVR-JANET RNN scan on 8 Trainium2 NeuronCores.

Strategy
--------
The recurrence h_{t+1} = f(h_t, x_t) over S=4096 steps is sequential, and at
B=128 / H=64 every per-step engine op is fixed-overhead dominated (ACT ~352cyc,
DVE ~151cyc of setup per instruction).  We therefore split the sequence into
NSEG segments processed in parallel (data-parallel over batch x segment), each
segment re-running a WARMUP window of the preceding steps from a zero state:
the sigmoid forget gate contracts initial-state error by ~0.967/step, so after
256 warmup steps the speculative state agrees with the exact state to ~5e-5
(validated against the fp32 reference in numpy).

Per-core layout is feature-major: state H = [h_I; h_Q] as [128 partitions,
F chains] tiles.  All linear algebra (including the h_I+h_Q sum, the dvr knot
weighting, H-G / H+G blends) rides on the TensorEngine via packed/stacked lhsT
weights; ACT does only Sin / Tanh / Tanh (one table set: silu_and_others —
sigmoid is rewritten as (1+tanh(z/2))/2); DVE does 5 fused elementwise ops per
step (two abs_max, one product, one scalar_tensor_tensor, one add).
"""

import os

import numpy as np

B, S, H, K = 128, 4096, 64, 4
NCORES = 8

# tunables
NSEG = 16          # parallel sequence segments (NSEG // NCORES groups per core)
WARMUP = 256       # speculative warmup steps per segment
CHUNK = 16         # steps of theta/mag data per input DMA
RING = 16          # state history ring (multiple of 4)

SEG_LEN = S // NSEG
GROUPS = NSEG // NCORES     # staggered groups per core
F = B                       # chains per group (full batch per segment)
GF = GROUPS * F
L = SEG_LEN + WARMUP        # loop steps (must be % 4 == 0 for the y windows)
assert L % 4 == 0 and RING % 4 == 0

_PROGRAM_CACHE = {}


def _pack_host(inputs):
    """All host-side packing: theta/mag precompute + lhsT weight matrices."""
    x = np.asarray(inputs["x"], np.float32)
    h_0 = np.asarray(inputs["h_0"], np.float32)
    cs = np.asarray(inputs["cs"], np.float32)
    knots = (np.arange(1, K + 1) / K).astype(np.float32)

    W_ph = np.asarray(inputs["W_ph"], np.float32)
    W_ptheta = np.asarray(inputs["W_ptheta"], np.float32)
    W_ah = np.asarray(inputs["W_ah"], np.float32)
    W_ax = np.asarray(inputs["W_ax"], np.float32)
    W_f = np.asarray(inputs["W_f"], np.float32)
    b_f = np.asarray(inputs["b_f"], np.float32)
    W_ccos = np.asarray(inputs["W_ccos"], np.float32)
    b_ccos = np.asarray(inputs["b_ccos"], np.float32)
    W_csin = np.asarray(inputs["W_csin"], np.float32)
    b_csin = np.asarray(inputs["b_csin"], np.float32)

    i_x = x[:, :, 0]
    q_x = x[:, :, 1]
    theta = np.arctan2(q_x, i_x).astype(np.float32)      # [B, S]
    mag = np.sqrt(i_x * i_x + q_x * q_x).astype(np.float32)

    I64 = np.eye(64, dtype=np.float32)
    I128 = np.eye(128, dtype=np.float32)

    def dup2x2(M):  # [64,64] -> [128,128] tiling (acts on h_I+h_Q, dup outputs)
        return np.tile(M, (2, 2)).astype(np.float32)

    c = cs
    sgn = np.sign(c).astype(np.float32)
    sgn[sgn == 0] = 1.0

    # dvr via relu identity (no |.| op on the DVE ISA for fp32):
    #   a = sum_k c_k |v - k_k| = sum_k 2 c_k relu(v - k_k) - sum_k c_k (v - k_k)
    # Stage-1 emits the shifted copies z_k = v - k_k; a single tensor_scalar
    # max(.,0) gives the relus; the 2c_k weighting (exact for negative c_k)
    # and the affine correction w both accumulate on the PE into psum_a.
    w = {}
    w["L1_th"] = dup2x2(W_ph.T)
    w["L1_f"] = dup2x2(W_f.T)
    w["L1_vA"] = dup2x2(W_ah.T)
    w["L1_vB"] = dup2x2(W_ah.T)
    Cs = float(c.sum())
    D = float((c * knots).sum())
    w["L_w"] = -Cs * dup2x2(W_ah.T)

    wth = W_ptheta[:, 0]
    wax = W_ax[:, 0]
    L1b_th = np.zeros((3, 128), np.float32)
    L1b_th[0, :64] = wth
    L1b_th[0, 64:] = wth
    w["L1b_th"] = L1b_th
    for nm, (ka, kb) in (
        ("L1b_vA", (knots[0], knots[1])),
        ("L1b_vB", (knots[2], knots[3])),
    ):
        Lb = np.zeros((3, 128), np.float32)
        Lb[1, :64] = wax
        Lb[1, 64:] = wax
        Lb[2, :64] = -ka
        Lb[2, 64:] = -kb
        w[nm] = Lb
    L1b_w = np.zeros((3, 128), np.float32)
    L1b_w[1, :] = -Cs * np.concatenate([wax, wax])
    L1b_w[2, :] = D
    w["L1b_w"] = L1b_w

    Idup = np.hstack([I64, I64]).astype(np.float32)  # [64,128]
    w["L_aA"] = np.vstack([2 * c[0] * Idup, 2 * c[1] * Idup])
    w["L_aB"] = np.vstack([2 * c[2] * Idup, 2 * c[3] * Idup])

    WLc, WRc = W_ccos[:, :64], W_ccos[:, 64:]
    WLs, WRs = W_csin[:, :64], W_csin[:, 64:]
    blk = np.zeros((128, 128), np.float32)
    blk[:64, :64] = WLc.T
    blk[64:, 64:] = WLs.T
    w["L2_H"] = blk.copy()
    blk = np.zeros((128, 128), np.float32)
    blk[:64, :64] = WRc.T
    blk[64:, 64:] = WRs.T
    w["L2_U"] = blk.copy()

    w["L_I"] = I128
    w["L_In"] = -I128
    w["L_Ih"] = 0.5 * I128

    # y projection: 4 step-slots x 2 components accumulated into one [8, F] psum
    wo1 = np.asarray(inputs["W_o1"], np.float32)[0, :]
    wo2 = np.asarray(inputs["W_o2"], np.float32)[0, :]
    for q in range(4):
        Ly = np.zeros((128, 8), np.float32)
        Ly[:64, 2 * q] = wo1
        Ly[64:, 2 * q + 1] = wo2
        w[f"L_y{q}"] = Ly

    w["bias_cs"] = np.concatenate(
        [np.full(64, np.pi / 2, np.float32), np.zeros(64, np.float32)]
    ).reshape(128, 1)
    w["bias_g"] = np.concatenate([b_ccos, b_csin]).astype(np.float32).reshape(128, 1)
    w["bias_f"] = (0.5 * np.concatenate([b_f, b_f])).astype(np.float32).reshape(128, 1)

    return theta, mag, h_0, w


def _seg_t0(seg, warmup):
    return 0 if seg == 0 else SEG_LEN * seg - warmup


def _pack_core_inputs(core, theta, mag, h_0, warmup, loop_len):
    """Per-core thm [3, L*GF] and hinit [128, GF] arrays."""
    thm = np.zeros((3, loop_len * GF), np.float32)
    hinit = np.zeros((128, GF), np.float32)
    for g in range(GROUPS):
        seg = GROUPS * core + g
        t0 = _seg_t0(seg, warmup)
        ts = np.clip(t0 + np.arange(loop_len), 0, S - 1)  # [L]
        th_g = theta[:, ts]  # [B, L]
        mg_g = mag[:, ts]
        for t in range(loop_len):
            off = t * GF + g * F
            thm[0, off : off + F] = th_g[:, t]
            thm[1, off : off + F] = mg_g[:, t]
            thm[2, off : off + F] = 1.0
        if seg == 0:
            h0T = h_0[0].T.astype(np.float32)  # [64, B]
            hinit[:64, g * F : (g + 1) * F] = h0T
            hinit[64:, g * F : (g + 1) * F] = h0T
    return thm, hinit


def build_program(w, loop_len=L, groups=GROUPS, f=F, chunk=CHUNK, ring=RING,
                  p1_bufs=1, p2_bufs=1, n_devices=NCORES):
    """Build + compile the SPMD Bass/Tile program. Returns (nc, names)."""
    import concourse.bass as bass
    import concourse.tile as tile
    import concourse.mybir as mybir
    from concourse import bacc

    fp32 = mybir.dt.float32
    AF = mybir.ActivationFunctionType
    OP = mybir.AluOpType
    gf = groups * f

    nc = bacc.Bacc(
        "TRN2", target_bir_lowering=False, debug=False, num_devices=n_devices
    )

    thm_in = nc.dram_tensor("thm", [3, loop_len * gf], fp32, kind="ExternalInput").ap()
    hinit_in = nc.dram_tensor("hinit", [128, gf], fp32, kind="ExternalInput").ap()
    wt_in = {
        k: nc.dram_tensor(f"w_{k}", list(v.shape), fp32, kind="ExternalInput").ap()
        for k, v in w.items()
    }
    y_out = nc.dram_tensor(
        "y", [groups * loop_len * 2, f], fp32, kind="ExternalOutput"
    ).ap()

    with tile.TileContext(nc) as tc:
        with (
            tc.tile_pool(name="wpool", bufs=1) as wpool,
            tc.tile_pool(name="hist", bufs=1) as hist_pool,
            tc.tile_pool(name="thm", bufs=2) as thm_pool,
            tc.tile_pool(name="work", bufs=2) as work,
            tc.tile_pool(name="p1", bufs=p1_bufs, space="PSUM") as p1_pool,
            tc.tile_pool(name="p2", bufs=p2_bufs, space="PSUM") as p2_pool,
            tc.tile_pool(name="py", bufs=1, space="PSUM") as py_pool,
        ):
            wt = {}
            for k, v in w.items():
                t = wpool.tile(list(v.shape), fp32, tag=f"w_{k}")
                nc.sync.dma_start(t[:], wt_in[k][:])
                wt[k] = t

            hist = []
            for g in range(groups):
                ht = hist_pool.tile([128, ring * f], fp32, tag=f"hist{g}")
                nc.sync.dma_start(ht[:, 0:f], hinit_in[:, g * f : (g + 1) * f])
                hist.append(ht)

            n_chunks = (loop_len + chunk - 1) // chunk
            thm_tiles = [None] * n_chunks

            # per-group persistent psum tiles (pool-managed, single buffer)
            p1 = [
                p1_pool.tile([128, 4 * f], fp32, tag=f"p1_{g}", name=f"p1_{g}")
                for g in range(groups)
            ]
            p2 = [
                p2_pool.tile([128, 4 * f], fp32, tag=f"p2_{g}", name=f"p2_{g}")
                for g in range(groups)
            ]
            py = [
                py_pool.tile([8, f], fp32, tag=f"py_{g}", name=f"py_{g}")
                for g in range(groups)
            ]

            mm = nc.tensor.matmul

            for t in range(loop_len):
                ci = t // chunk
                if t % chunk == 0:
                    th_t = thm_pool.tile([3, chunk * gf], fp32, tag="thm")
                    nc.sync.dma_start(
                        th_t[:], thm_in[:, t * gf : (t + chunk) * gf]
                    )
                    thm_tiles[ci] = th_t

                for g in range(groups):
                    Hap = hist[g][:, (t % ring) * f : (t % ring + 1) * f]
                    Hnap = hist[g][:, ((t + 1) % ring) * f : ((t + 1) % ring + 1) * f]
                    thm_sl = thm_tiles[ci][
                        :, (t % chunk) * gf + g * f : (t % chunk) * gf + (g + 1) * f
                    ]
                    th_sl = p1[g][:, 0:f]
                    vA_sl = p1[g][:, f : 2 * f]
                    vB_sl = p1[g][:, 2 * f : 3 * f]
                    f_sl = p1[g][:, 3 * f : 4 * f]
                    a_sl = p2[g][:, 0:f]
                    g_sl = p2[g][:, f : 2 * f]
                    R_sl = p2[g][:, 2 * f : 3 * f]
                    P_sl = p2[g][:, 3 * f : 4 * f]

                    # stage 1: [theta_t; theta_t], [c1(v-k1); c2(v-k2)],
                    #          [c3(v-k3); c4(v-k4)], [fz; fz]
                    mm(th_sl, wt["L1_th"][:], Hap, start=True, stop=False)
                    mm(th_sl, wt["L1b_th"][:], thm_sl, start=False, stop=True)
                    mm(vA_sl, wt["L1_vA"][:], Hap, start=True, stop=False)
                    mm(vA_sl, wt["L1b_vA"][:], thm_sl, start=False, stop=True)
                    mm(vB_sl, wt["L1_vB"][:], Hap, start=True, stop=False)
                    mm(vB_sl, wt["L1b_vB"][:], thm_sl, start=False, stop=True)
                    mm(f_sl, wt["L1_f"][:], Hap, start=True, stop=True)

    # CS = [cos(theta_t); sin(theta_t)] via Sin(x + [pi/2; 0])
                    CS = work.tile([128, f], fp32, tag=f"cs{g}")
                    nc.scalar.activation(
                        CS[:], th_sl, AF.Sin, bias=wt["bias_cs"][:, 0:1]
                    )
                    # relu of all four shifted copies in one op: [vA | vB]
                    SAB = work.tile([128, 2 * f], fp32, tag=f"sab{g}")
                    nc.vector.tensor_scalar(
                        SAB[:], p1[g][:, f : 3 * f], 0.0, None, OP.max
                    )

                    # a = sum_k 2 c_k relu(z_k) + w, duplicated on both halves
                    mm(a_sl, wt["L_aA"][:], SAB[:, 0:f], start=True, stop=False)
                    mm(a_sl, wt["L_aB"][:], SAB[:, f : 2 * f], start=False, stop=False)
                    mm(a_sl, wt["L_w"][:], Hap, start=False, stop=False)
                    mm(a_sl, wt["L1b_w"][:], thm_sl, start=False, stop=True)

                    U = work.tile([128, f], fp32, tag=f"u{g}")
                    nc.vector.tensor_mul(U[:], a_sl, CS[:])

                    # stage 2: g_pre = [WLc @ h_I + WRc @ u_cos; sin half]
                    mm(g_sl, wt["L2_H"][:], Hap, start=True, stop=False)
                    mm(g_sl, wt["L2_U"][:], U[:], start=False, stop=True)

                    Gt = work.tile([128, f], fp32, tag=f"g{g}")
                    nc.scalar.activation(
                        Gt[:], g_sl, AF.Tanh, bias=wt["bias_g"][:, 0:1]
                    )
                    Tf = work.tile([128, f], fp32, tag=f"tf{g}")
                    nc.scalar.activation(
                        Tf[:], f_sl, AF.Tanh, bias=wt["bias_f"][:, 0:1], scale=0.5
                    )

                    # R = H - G ; P = (H + G) / 2  (on PE)
                    mm(R_sl, wt["L_I"][:], Hap, start=True, stop=False)
                    mm(R_sl, wt["L_In"][:], Gt[:], start=False, stop=True)
                    mm(P_sl, wt["L_Ih"][:], Hap, start=True, stop=False)
                    mm(P_sl, wt["L_Ih"][:], Gt[:], start=False, stop=True)

                    # Q = 0.5 * Tf * R ;  H' = P + Q
                    Q = work.tile([128, f], fp32, tag=f"q{g}")
                    nc.vector.scalar_tensor_tensor(
                        Q[:], Tf[:], 0.5, R_sl, OP.mult, OP.mult
                    )
                    nc.vector.tensor_add(Hnap, Q[:], P_sl)

                    # y_t = [W_o1 @ h_I'; W_o2 @ h_Q'] into step-slot rows of py
                    mm(
                        py[g][:],
                        wt[f"L_y{t % 4}"][:],
                        Hnap,
                        start=(t % 4 == 0),
                        stop=(t % 4 == 3),
                    )
                    if t % 4 == 3:
                        y_sb = work.tile([8, f], fp32, tag=f"ysb{g}")
                        nc.vector.tensor_copy(y_sb[:], py[g][:])
                        row0 = (g * loop_len + t - 3) * 2
                        nc.sync.dma_start(
                            y_out[row0 : row0 + 8, :], y_sb[:]
                        )

    nc.compile()
    return nc


TRACE = bool(int(os.environ.get("DJANET_TRACE", "0")))
LAST_EXEC_NS = None
LAST_RESULTS = None


def kernel(**inputs):
    global LAST_EXEC_NS, LAST_RESULTS
    from concourse.bass_utils import run_bass_kernel_spmd

    theta, mag, h_0, w = _pack_host(inputs)

    key = "main"
    if key not in _PROGRAM_CACHE:
        _PROGRAM_CACHE[key] = build_program(w)
    nc = _PROGRAM_CACHE[key]

    in_maps = []
    for core in range(NCORES):
        thm, hinit = _pack_core_inputs(core, theta, mag, h_0, WARMUP, L)
        m = {"thm": thm, "hinit": hinit}
        for k, v in w.items():
            m[f"w_{k}"] = v
        in_maps.append(m)

    kwargs = {}
    if TRACE:
        kwargs = dict(trace=True)
    import time as _time

    _t0 = _time.time()
    res = run_bass_kernel_spmd(nc, in_maps, list(range(NCORES)), **kwargs)
    globals()["LAST_RUN_WALL_S"] = _time.time() - _t0
    LAST_EXEC_NS = res.exec_time_ns
    LAST_RESULTS = res

    b_o1 = float(np.asarray(inputs["b_o1"], np.float32)[0])
    b_o2 = float(np.asarray(inputs["b_o2"], np.float32)[0])

    out = np.zeros((B, S, 2), np.float32)
    for core in range(NCORES):
        y = res.results[core]["y"].reshape(GROUPS, L, 2, F)
        for g in range(GROUPS):
            seg = GROUPS * core + g
            tau0 = 0 if seg == 0 else WARMUP
            sl = y[g, tau0 : tau0 + SEG_LEN]  # [SEG_LEN, 2, B]
            out[:, SEG_LEN * seg : SEG_LEN * (seg + 1), 0] = sl[:, 0, :].T + b_o1
            out[:, SEG_LEN * seg : SEG_LEN * (seg + 1), 1] = sl[:, 1, :].T + b_o2
    return out
